# revision 11
# baseline (speedup 1.0000x reference)
"""DEDICOM decoder forward on 8 Trainium2 NeuronCores.

score = sigmoid((z_i * (z_j @ R.T)) @ (D*D).T)

Fast path (used when all rows of D are identical, as in the reference where
D = ones): every output column is the same value
    s[b] = sum_h (z_i[b,h] * D0[h]^2) * (z_j @ R.T)[b,h]
so the kernel computes one sigmoid column per core and the host broadcasts it
to [B, 960]. This removes the [B,960] GEMM and the 60 MB of output DMA.

Per-core fast-path dataflow (4096 rows):
  - inputs in fp16 (halves HBM traffic; fp16's 10-bit mantissa keeps the
    error in the same class as the f32r baseline)
  - MM1: psum[b,h'] = sum_h z_jT[h,b] * R.T[h,h']   (stationary = z_jT
    128-row batch chunks, moving = R.T, N=512) -> PSUM in [b, h'] layout
  - DVE tensor_tensor_reduce: s[b] = sum_h' psum[b,h'] * z_i'[b,h']
    (one fused multiply+reduce per 128-row chunk)
  - one sigmoid on ACT over [128, 32], one 16 KB output DMA

Generic path (any D): the original data-parallel kernel with both GEMMs.
"""
import sys

sys.path.insert(0, "/opt/trn_rl_repo")

import numpy as np  # noqa: E402

B = 32768
H = 512  # hidden
R_SE = 960  # num relation types
N_CORES = 8
BS = B // N_CORES  # 4096 batch rows per core
NK = H // 128  # 4 h-chunks
NCH = BS // 128  # 32 batch chunks of 128 per core

_compiled_fast = None
_compiled_generic = None


def _build_fast(dt16_name="float16", use_ttr=False, warm_mms=32):
    import concourse.tile as tile
    import concourse.mybir as mybir
    from concourse import bacc

    f32 = mybir.dt.float32
    f16 = getattr(mybir.dt, dt16_name)
    mult = mybir.AluOpType.mult
    add = mybir.AluOpType.add

    nc = bacc.Bacc("TRN2", target_bir_lowering=False, debug=False)
    zi_d = nc.dram_tensor("zi", [BS, H], f16, kind="ExternalInput").ap()  # z_i * D0^2
    zjt_d = nc.dram_tensor("zjt", [H, BS], f16, kind="ExternalInput").ap()  # z_j.T
    rt_d = nc.dram_tensor("rt", [H, H], f16, kind="ExternalInput").ap()  # R.T
    out_d = nc.dram_tensor("out", [128, NCH], f32, kind="ExternalOutput").ap()

    with tile.TileContext(nc) as tc:
        with (
            tc.tile_pool(name="const", bufs=1) as const,
            tc.tile_pool(name="zjt", bufs=3) as zjtp,
            tc.tile_pool(name="zi", bufs=3) as zip_,
            tc.tile_pool(name="scr", bufs=2) as scrp,
            tc.tile_pool(name="ps", bufs=6, space="PSUM") as psp,
            tc.tile_pool(name="warm", bufs=1, space="PSUM") as warmp,
        ):
            # PE warmup while the first DMAs land: short junk matmuls keep the
            # HAM activity window busy so the clock gate lifts ASAP. Small N
            # so a late junk MM delays the first real MM by <60ns.
            warm_f = const.tile([128, 128], f32, tag="warm_f")
            nc.vector.memset(warm_f[:], 0.0)
            warm_sb = const.tile([128, 128], f16, tag="warm_sb")
            nc.vector.tensor_copy(warm_sb[:], warm_f[:])
            warm_ps = warmp.tile([128, 64], f32, tag="warm_ps")
            for _ in range(warm_mms):
                nc.tensor.matmul(
                    warm_ps[:], warm_sb[:], warm_sb[:, :64], start=True, stop=True
                )
            # preload the sigmoid ACT table during the DMA head wait so the
            # final sigmoid doesn't pay the table-load latency
            warm_sg = const.tile([128, 1], f32, tag="warm_sg")
            nc.scalar.activation(
                warm_sg[:], warm_f[:, 0:1], mybir.ActivationFunctionType.Sigmoid
            )

            rt_r = const.tile([128, NK, H], f16, tag="rt_r")
            scol = const.tile([128, NCH], f32, tag="scol")

            sizes = [256, 256] + [512] * 7
            offs = [sum(sizes[:i]) for i in range(len(sizes))]
            for t, (b0, bt) in enumerate(zip(offs, sizes)):
                ncch = bt // 128
                zjt_t = zjtp.tile([128, NK, bt], f16, tag="zjt", name=f"zjt_{t}")
                if t == 0:
                    # per-k-chunk DMAs so the first matmul only waits for the
                    # k=0 slice (64KB) + rt k=0, interleaved in that order
                    for k in range(NK):
                        nc.sync.dma_start(
                            zjt_t[:, k, :],
                            zjt_d[k * 128 : (k + 1) * 128, b0 : b0 + bt],
                        )
                        nc.sync.dma_start(
                            rt_r[:, k, :], rt_d[k * 128 : (k + 1) * 128, :]
                        )
                else:
                    nc.sync.dma_start(
                        zjt_t[:],
                        zjt_d[:, b0 : b0 + bt].rearrange("(k p) b -> p k b", p=128),
                    )
                # zi rides the SWDGE (gpsimd) path so it never queues behind
                # the zjt stream on the sync HWDGE ring
                zi_t = zip_.tile([128, ncch, H], f16, tag="zi", name=f"zi_{t}")
                nc.gpsimd.dma_start(
                    zi_t[:],
                    zi_d[b0 : b0 + bt, :].rearrange("(c p) h -> p c h", p=128),
                )
                for c in range(ncch):
                    cg = b0 // 128 + c
                    ps = psp.tile([128, H], f32, tag="ps", name=f"ps_{cg}")
                    for k in range(NK):
                        nc.tensor.matmul(
                            ps[:],
                            zjt_t[:, k, c * 128 : (c + 1) * 128],
                            rt_r[:, k, :],
                            start=(k == 0),
                            stop=(k == NK - 1),
                        )
                    # DVE multiply (fp16 out: products round to fp16, which is
                    # negligible vs the fp16 input rounding), then ACT reduce
                    # via activation accumulate — fp16 input gives ACT 2x rate
                    sc = scrp.tile([128, H], f16, tag="sc", name=f"sc_{cg}")
                    if use_ttr:
                        nc.vector.tensor_tensor_reduce(
                            out=sc[:],
                            in0=ps[:],
                            in1=zi_t[:, c, :],
                            scale=1.0,
                            scalar=0.0,
                            op0=mult,
                            op1=add,
                            accum_out=scol[:, cg : cg + 1],
                        )
                    else:
                        nc.vector.tensor_mul(sc[:], ps[:], zi_t[:, c, :])
                        sc2 = scrp.tile([128, H], f16, tag="sc2", name=f"sc2_{cg}")
                        nc.scalar.activation(
                            sc2[:],
                            sc[:],
                            mybir.ActivationFunctionType.Copy,
                            accum_out=scol[:, cg : cg + 1],
                        )

            sig = const.tile([128, NCH], f32, tag="sig")
            nc.scalar.activation(
                sig[:], scol[:], mybir.ActivationFunctionType.Sigmoid
            )
            nc.sync.dma_start(out_d, sig[:])

    nc.compile()
    return nc


def _get_fast():
    global _compiled_fast
    if _compiled_fast is None:
        _compiled_fast = _build_fast()
    return _compiled_fast


def _kernel_fast(z_i, z_j, R, D):
    from concourse import bass_utils

    nc = _get_fast()

    d2 = (D[0].astype(np.float32)) ** 2
    if np.all(d2 == 1.0):
        zi_s = z_i.astype(np.float16)
    else:
        zi_s = (z_i.astype(np.float32) * d2[None, :]).astype(np.float16)
    zjt = np.ascontiguousarray(z_j.astype(np.float16).T)
    rt = np.ascontiguousarray(R.astype(np.float16).T)

    in_maps = []
    for c in range(N_CORES):
        sl = slice(c * BS, (c + 1) * BS)
        in_maps.append(
            {
                "zi": np.ascontiguousarray(zi_s[sl]),
                "zjt": np.ascontiguousarray(zjt[:, sl]),
                "rt": rt,
            }
        )

    res = bass_utils.run_bass_kernel_spmd(nc, in_maps, core_ids=list(range(N_CORES)))
    global last_result
    last_result = res
    col = np.empty(B, dtype=np.float32)
    for c in range(N_CORES):
        arr = np.asarray(res.results[c]["out"], dtype=np.float32)  # [128, NCH]
        col[c * BS : (c + 1) * BS] = arr.T.ravel()
    out = np.empty((B, R_SE), dtype=np.float32)
    out[:] = col[:, None]
    return out


# ---------------------------------------------------------------------------
# Generic path (any D): original data-parallel kernel with both GEMMs.
# ---------------------------------------------------------------------------

BT = 512  # batch tile
NT = BS // BT  # 8 batch tiles per core
RH = R_SE // 2  # 480, moving-dim half for MM2


def _build_generic():
    import concourse.tile as tile
    import concourse.mybir as mybir
    from concourse import bacc

    f32 = mybir.dt.float32
    f32r = mybir.dt.float32r

    nc = bacc.Bacc("TRN2", target_bir_lowering=False, debug=False)
    zit_d = nc.dram_tensor("zit", [H, BS], f32, kind="ExternalInput").ap()
    zjt_d = nc.dram_tensor("zjt", [H, BS], f32r, kind="ExternalInput").ap()
    rt_d = nc.dram_tensor("rt", [H, H], f32r, kind="ExternalInput").ap()  # R.T
    d2t_d = nc.dram_tensor("d2t", [H, R_SE], f32r, kind="ExternalInput").ap()
    out_d = nc.dram_tensor("out", [BS, R_SE], f32, kind="ExternalOutput").ap()

    with tile.TileContext(nc) as tc:
        with (
            tc.tile_pool(name="const", bufs=1) as const,
            tc.tile_pool(name="zt", bufs=4) as ztp,
            tc.tile_pool(name="qp", bufs=2) as qp,
            tc.tile_pool(name="sig", bufs=6) as sigp,
            tc.tile_pool(name="ps1", bufs=3, space="PSUM") as ps1p,
            tc.tile_pool(name="ps2", bufs=4, space="PSUM") as ps2p,
            tc.tile_pool(name="warm", bufs=1, space="PSUM") as warmp,
        ):
            rt_r = const.tile([128, NK, H], f32r, tag="rt_r")
            nc.sync.dma_start(rt_r[:], rt_d.rearrange("(k p) n -> p k n", p=128))

            warm_f = const.tile([128, BT], f32, tag="warm_f")
            nc.vector.memset(warm_f[:], 0.0)
            warm_sb = const.tile([128, BT], f32r, tag="warm_sb")
            nc.vector.tensor_copy(warm_sb[:], warm_f[:])
            warm_ps = warmp.tile([128, BT], f32, tag="warm_ps")
            for _ in range(10):
                nc.tensor.matmul(
                    warm_ps[:], warm_sb[:, :128], warm_sb[:], start=True, stop=True
                )

            d2t_r = const.tile([128, NK, R_SE], f32r, tag="d2t_r")

            sizes = [256, 256] + [512] * (NT - 1)
            offs = [sum(sizes[:i]) for i in range(len(sizes))]
            tiles = list(zip(offs, sizes))
            for t, (b0, bt) in enumerate(tiles):
                nm = bt // 128
                zjt_r = ztp.tile([128, NK, bt], f32r, tag="zjt", name=f"zjt_{t}")
                nc.sync.dma_start(
                    zjt_r[:],
                    zjt_d[:, b0 : b0 + bt].rearrange("(k p) b -> p k b", p=128),
                )
                if t == 0:
                    nc.sync.dma_start(
                        d2t_r[:, :, 0:RH],
                        d2t_d[:, 0:RH].rearrange("(k p) n -> p k n", p=128),
                    )
                zit_f = ztp.tile([128, NK, bt], f32, tag="zit", name=f"zit_{t}")
                nc.sync.dma_start(
                    zit_f[:],
                    zit_d[:, b0 : b0 + bt].rearrange("(k p) b -> p k b", p=128),
                )
                if t == 0:
                    nc.sync.dma_start(
                        d2t_r[:, :, RH:R_SE],
                        d2t_d[:, RH:R_SE].rearrange("(k p) n -> p k n", p=128),
                    )

                q_r = qp.tile([128, NK, bt], f32r, tag="q", name=f"q_{t}")
                for j in range(NK):
                    p1 = ps1p.tile([128, bt], f32, tag="ps1", name=f"p1_{t}_{j}")
                    for k in range(NK):
                        nc.tensor.matmul(
                            p1[:],
                            rt_r[:, k, j * 128 : (j + 1) * 128],
                            zjt_r[:, k, :],
                            start=(k == 0),
                            stop=(k == NK - 1),
                        )
                    nc.vector.tensor_mul(q_r[:, j, :], p1[:], zit_f[:, j, :])

                last_tile = t == len(tiles) - 1
                for m in range(nm):
                    sg = sigp.tile([128, R_SE], f32, tag="sg", name=f"sg_{t}_{m}")
                    for rh in range(2):
                        p2 = ps2p.tile([128, RH], f32, tag="ps2", name=f"p2_{t}_{m}_{rh}")
                        for k in range(NK):
                            nc.tensor.matmul(
                                p2[:],
                                q_r[:, k, m * 128 : (m + 1) * 128],
                                d2t_r[:, k, rh * RH : (rh + 1) * RH],
                                start=(k == 0),
                                stop=(k == NK - 1),
                            )
                        nc.scalar.activation(
                            sg[:, rh * RH : (rh + 1) * RH],
                            p2[:],
                            mybir.ActivationFunctionType.Sigmoid,
                        )
                        if last_tile:
                            nc.sync.dma_start(
                                out_d[
                                    b0 + m * 128 : b0 + (m + 1) * 128,
                                    rh * RH : (rh + 1) * RH,
                                ],
                                sg[:, rh * RH : (rh + 1) * RH],
                            )
                    if not last_tile:
                        nc.gpsimd.dma_start(
                            out_d[b0 + m * 128 : b0 + (m + 1) * 128, :], sg[:]
                        )

    nc.compile()
    return nc


def _get_generic():
    global _compiled_generic
    if _compiled_generic is None:
        _compiled_generic = _build_generic()
    return _compiled_generic


def _round_f32r(x: np.ndarray) -> np.ndarray:
    """Round fp32 to the f32r grid (12 dropped mantissa bits, round-nearest)."""
    b = np.ascontiguousarray(x, dtype=np.float32).view(np.uint32)
    r = (b + 0x800 + ((b >> 12) & 1)) & np.uint32(0xFFFFF000)
    return r.view(np.float32)


def _kernel_generic(z_i, z_j, R, D):
    from concourse import bass_utils

    nc = _get_generic()

    zit = np.ascontiguousarray(z_i.T)  # [H, B]
    zjt = _round_f32r(np.ascontiguousarray(z_j.T))
    rt = _round_f32r(np.asarray(R, dtype=np.float32).T)
    d2 = np.asarray(D, dtype=np.float32)
    d2t = _round_f32r((d2 * d2).T)

    in_maps = []
    for c in range(N_CORES):
        sl = slice(c * BS, (c + 1) * BS)
        in_maps.append(
            {
                "zit": np.ascontiguousarray(zit[:, sl]),
                "zjt": np.ascontiguousarray(zjt[:, sl]),
                "rt": rt,
                "d2t": d2t,
            }
        )

    res = bass_utils.run_bass_kernel_spmd(nc, in_maps, core_ids=list(range(N_CORES)))
    global last_result
    last_result = res
    out = np.empty((B, R_SE), dtype=np.float32)
    for c in range(N_CORES):
        out[c * BS : (c + 1) * BS] = res.results[c]["out"]
    return out


def kernel(z_i: np.ndarray, z_j: np.ndarray, R: np.ndarray, D: np.ndarray, **extra):
    z_i = np.asarray(z_i, dtype=np.float32)
    z_j = np.asarray(z_j, dtype=np.float32)
    R = np.asarray(R, dtype=np.float32)
    D = np.asarray(D, dtype=np.float32)

    # Fast path requires every row of D identical (so all output columns are
    # equal) and values comfortably inside fp16 range.
    rows_same = bool(np.all(D == D[0]))
    d2 = D[0].astype(np.float32) ** 2
    in_range = bool(
        np.all(np.isfinite(z_i))
        and np.all(np.isfinite(z_j))
        and np.all(np.isfinite(R))
        and np.abs(z_i * d2[None, :] if not np.all(d2 == 1.0) else z_i).max() < 6.0e4
        and np.abs(z_j).max() < 6.0e4
        and np.abs(R).max() < 6.0e4
    )
    if rows_same and in_range:
        return _kernel_fast(z_i, z_j, R, D)
    return _kernel_generic(z_i, z_j, R, D)


last_result = None


def _install_ntff_shim():
    """Provide antenv.axon_hooks (absent from this image) so that
    run_bass_kernel_spmd(trace=True) can capture NTFF profiles through
    the axon PJRT .so. No-op if anything is missing."""
    import types
    import contextlib
    import ctypes

    try:
        import antenv
        import antenv.axon_hooks  # noqa: F401

        return  # already present
    except ImportError:
        pass

    so_path = "/opt/axon/libaxon_pjrt.so"
    try:
        lib = ctypes.CDLL(so_path)
    except OSError:
        return
    if not hasattr(lib, "axon_start_nrt_profile"):
        return
    lib.axon_start_nrt_profile.argtypes = [
        ctypes.POINTER(ctypes.c_int64),
        ctypes.c_size_t,
    ]
    lib.axon_start_nrt_profile.restype = ctypes.c_int64
    lib.axon_stop_nrt_profile.argtypes = [ctypes.c_char_p]
    lib.axon_stop_nrt_profile.restype = ctypes.c_int64

    @contextlib.contextmanager
    def _hook(output_dir, device_ids):
        import jax

        jax.devices()
        if device_ids:
            ids = (ctypes.c_int64 * len(device_ids))(*device_ids)
            rc = lib.axon_start_nrt_profile(ids, len(device_ids))
        else:
            rc = lib.axon_start_nrt_profile(None, 0)
        if rc != 0:
            raise RuntimeError(f"axon_start_nrt_profile rc={rc}")
        try:
            yield
        finally:
            n = lib.axon_stop_nrt_profile(str(output_dir).encode())
            print(f"ntff profile: {n} file(s) written to {output_dir}", file=sys.stderr)

    mod = types.ModuleType("antenv.axon_hooks")
    mod.get_axon_ntff_profile_hook = lambda: _hook
    mod.set_axon_ntff_profile_hook = lambda h: None
    sys.modules["antenv.axon_hooks"] = mod
    antenv.axon_hooks = mod


_install_ntff_shim()
